# revision 1
# baseline (speedup 1.0000x reference)
"""Trainium2 Bass kernel for nn_MultiDiscretePolicy.

Math:
  h   = relu(s @ W1 + b1)                         [B, 1024]
  aw  = h @ W2 + b2                               [B, 256]
  d   = aw @ Wd + db    (Wd = head_W[...,0]-head_W[...,1] transposed)
  out pair h: even = 1.0 if (logit0+g0) >= (logit1+g1) else 0.0, odd = 1-even
The reference's y + stop_grad(y_hard - y) is exactly one-hot in fp32, and
argmax(softmax(z)) == argmax(z), so the output reduces to the sign test
  even = (d >= gdn),  gdn = q0 - q1,  q_k = log(-log(u_k + EPS) + EPS)
(two Ln passes on ACT; all-Ln keeps a single activation table set loaded).

Sharding: pure data parallel over the batch dim across 8 cores.  Matmuls keep
features on PSUM partitions / batch on the moving free dim, so the only
transpose needed (s^T) is done on host.  f32r matmuls (12-bit mantissa
products, fp32 accumulate) at full PE rate.
"""
from contextlib import ExitStack

import numpy as np

import concourse.bass as bass
import concourse.mybir as mybir
import concourse.tile as tile
from concourse import bacc
from concourse import bass_utils
from concourse.bass import ts, ds

N_CORES = 8
B, S_DIM, H_DIM, A_DIM = 32768, 1024, 1024, 512
D_HEAD = A_DIM // 2
EPS = 1e-20
BB = 512           # batch columns per block (one PSUM bank of fp32)
f32 = mybir.dt.float32
f32r = mybir.dt.float32r
AFT = mybir.ActivationFunctionType
OP = mybir.AluOpType

LAST_EXEC_NS = None

_cache: dict = {}


def _build(rpc: int, has_db: bool, loop_iters: int | None = None):
    """Build the per-core kernel for `rpc` batch rows per core.

    loop_iters: timing-only mode — repeat the whole pass that many times
    inside a hardware For_i loop (same data each iteration).
    """
    nb = rpc // BB
    nc = bacc.Bacc("TRN2", target_bir_lowering=False, debug=False,
                   num_devices=N_CORES)

    sTd = nc.dram_tensor("sT", [S_DIM, rpc], f32r, kind="ExternalInput").ap()
    u2d = nc.dram_tensor("u2", [rpc, 2 * A_DIM], f32, kind="ExternalInput").ap()
    W1d = nc.dram_tensor("W1d", [S_DIM, H_DIM], f32r, kind="ExternalInput").ap()
    # W2/Wd arrive host-packed partition-major so one partition's data is a
    # single contiguous run (DMA chunk size drives HBM efficiency)
    W2d = nc.dram_tensor("W2d", [128, 8 * D_HEAD], f32r, kind="ExternalInput").ap()
    Wdd = nc.dram_tensor("Wdd", [128, 2 * A_DIM], f32r, kind="ExternalInput").ap()
    b1d = nc.dram_tensor("b1d", [128, 8], f32, kind="ExternalInput").ap()
    b2d = nc.dram_tensor("b2d", [128, 2], f32, kind="ExternalInput").ap()
    if has_db:
        dbd = nc.dram_tensor("dbd", [1, A_DIM], f32r, kind="ExternalInput").ap()
    # only the even elements of each output pair are shipped (odd = 1 - even),
    # as uint8 {0,1} — exact, since the fp32 output is exactly one-hot
    outd = nc.dram_tensor("out", [rpc, A_DIM], mybir.dt.uint8,
                          kind="ExternalOutput").ap()

    sTv = sTd.rearrange("(a p) b -> p a b", p=128)      # [128, 8, rpc]
    # u arrives host-permuted in row pairs: row = q*256 + p*2 + h, so each
    # partition line of a pair-load is 8KB contiguous
    u2v = u2d.rearrange("(q p h) m -> p q (h m)", p=128, h=2)
    # out leaves partition-major within each 2-block group: DRAM row
    # g*(128*x) + p*x + xx, so each group store writes x*512 contiguous
    # bytes per partition (host undoes the permutation)
    xgrp = 8 if nb % 2 == 0 else 4
    assert nb % 2 == 0 or nb == 1
    outv = outd.rearrange("(g p x) m -> p g x m", p=128, x=xgrp)

    with tile.TileContext(nc) as tc, ExitStack() as ctx:
        wp = ctx.enter_context(tc.tile_pool(name="weights", bufs=1))
        sT_pool = ctx.enter_context(tc.tile_pool(name="sTp", bufs=4))
        u_pool = ctx.enter_context(tc.tile_pool(name="up", bufs=4))
        p_pool = ctx.enter_context(tc.tile_pool(name="pp", bufs=2))
        q_pool = ctx.enter_context(tc.tile_pool(name="qp", bufs=2))
        gdn_pool = ctx.enter_context(tc.tile_pool(name="gdnp", bufs=6))
        hT_pool = ctx.enter_context(tc.tile_pool(name="hTp", bufs=10))
        awT_pool = ctx.enter_context(tc.tile_pool(name="awTp", bufs=3))
        out_pool = ctx.enter_context(tc.tile_pool(name="outp", bufs=3))
        # h and d tiles share one 6-slot tag so mm1 (block0's k-outer) and
        # mm3 time-share PSUM banks; awT holds the other 2 banks.
        hd_psp = ctx.enter_context(tc.tile_pool(name="hdps", bufs=6, space="PSUM"))
        a_psp = ctx.enter_context(tc.tile_pool(name="aps", bufs=2, space="PSUM"))

        W1v = W1d.rearrange("(a p) j -> p a j", p=128)
        if loop_iters is not None:
            loop_cm = tc.For_i(0, loop_iters, 1)
            loop_cm.__enter__()
        # sT is processed in 2-block groups so each DMA's partition line is
        # a 4KB contiguous run; group 0 is loaded chunk-by-chunk interleaved
        # with W1 so block0's k-outer matmuls track the DMA stream.
        ngrp = (nb + 1) // 2

        def g_cols(g):
            return min(2 * BB, rpc - g * 2 * BB)

        def sT_group_load(g, split):
            tiles = []
            for ka in range(2):
                sT_t = sT_pool.tile([128, 4, 2 * BB], f32r, name="sT_t")
                if not split:
                    nc.sync.dma_start(
                        sT_t[:, :, 0:g_cols(g)],
                        sTv[:, ka * 4:(ka + 1) * 4, ds(g * 2 * BB, g_cols(g))])
                tiles.append(sT_t)
            return tiles

        W1_sb = wp.tile([128, 8, H_DIM], f32r)
        g0_ts = sT_group_load(0, split=True)
        for k in range(8):
            if k == 0:
                nc.sync.dma_start(W1_sb[:, 0, 0:768], W1v[:, 0, 0:768])
            else:
                nc.sync.dma_start(W1_sb[:, k, :], W1v[:, k, :])
            if k == 0 and g_cols(0) > BB:
                # split block0's first chunk so the very first matmul is
                # gated on ~0.6MB of DMA instead of ~1MB
                nc.sync.dma_start(g0_ts[0][:, 0, 0:BB], sTv[:, 0, ds(0, BB)])
                nc.sync.dma_start(g0_ts[0][:, 0, BB:g_cols(0)],
                                  sTv[:, 0, ds(BB, g_cols(0) - BB)])
            else:
                nc.sync.dma_start(g0_ts[k // 4][:, k % 4, 0:g_cols(0)],
                                  sTv[:, k, ds(0, g_cols(0))])
            if k == 0:
                nc.sync.dma_start(W1_sb[:, 0, 768:1024], W1v[:, 0, 768:1024])
        b1_sb = wp.tile([128, 8], f32)
        nc.sync.dma_start(b1_sb[:], b1d[:])
        W2_sb = wp.tile([128, 8 * D_HEAD], f32r)
        nc.sync.dma_start(W2_sb[:], W2d[:])
        W2_v = W2_sb.rearrange("p (j d) -> p j d", j=8)
        Wd_sb = wp.tile([128, 2 * A_DIM], f32r)
        nc.sync.dma_start(Wd_sb[:], Wdd[:])
        Wd_v = Wd_sb.rearrange("p (a m) -> p a m", a=2)
        b2_sb = wp.tile([128, 2], f32)
        nc.sync.dma_start(b2_sb[:], b2d[:])
        eps_sb = wp.tile([128, 1], f32)
        nc.vector.memset(eps_sb[:], EPS)
        if has_db:
            db_sb = wp.tile([1, A_DIM], f32r)
            nc.sync.dma_start(db_sb[:], dbd[:])
            ones_sb = wp.tile([1, 128], f32r)
            nc.vector.memset(ones_sb[:].bitcast(f32), 1.0)

        cur_sT = g0_ts
        next_sT = None
        o_t = None

        for b0 in range(nb):
            g = b0 // 2
            bw = b0 % 2
            if bw == 0:
                o_t = out_pool.tile([128, 8, A_DIM], mybir.dt.uint8,
                                    name="o_t")
            else:
                # prefetch the NEXT 2-block sT group one block ahead — late
                # enough to keep the head DMA queue short, early enough that
                # the 4MB lands within one block period
                if g + 1 < ngrp:
                    next_sT = sT_group_load(g + 1, split=False)
            # this block's u row-pairs (each an 8KB-line 1MB load)
            up_ts = []
            for q in range(2):
                u_t = u_pool.tile([128, 2, 2 * A_DIM], f32, name="u_t")
                nc.sync.dma_start(
                    u_t[:].rearrange("p h m -> p (h m)"),
                    u2v[:, b0 * 2 + q, :])
                up_ts.append(u_t)
            u_ts = [up_ts[bs // 2][:, bs % 2, :] for bs in range(4)]

            def sT_at(k):
                return cur_sT[k // 4][:, k % 4, ds(bw * BB, BB)]

            # ---- gumbel: p = ln(u+eps); q = ln(-p+eps); gdn = q0-q1 ----
            # (for block 0 this is emitted after the matmuls: u arrives late
            # and the Lns must not block the relus in the ACT FIFO)
            gdn_ts = []
            q_ts = []

            def ln_part(bs):
                p_t = p_pool.tile([128, 2 * A_DIM], f32, name="p_t")
                nc.scalar.activation(p_t[:], u_ts[bs], AFT.Ln,
                                     bias=eps_sb[:], scale=1.0)
                q_t = q_pool.tile([128, 2 * A_DIM], f32, name="q_t")
                nc.scalar.activation(q_t[:], p_t[:], AFT.Ln,
                                     bias=eps_sb[:], scale=-1.0)
                q_ts.append(q_t)

            def gdn_part(bs):
                q_t = q_ts[bs]
                gdn_t = gdn_pool.tile([128, A_DIM], f32, name="gdn_t")
                nc.vector.tensor_tensor(gdn_t[:], q_t[:, 0::2], q_t[:, 1::2],
                                        OP.subtract)
                gdn_ts.append(gdn_t)

            def gumbel(bs):
                ln_part(bs)
                gdn_part(bs)

            if b0 > 0:
                # Lns go to the ACT queue now; the gdn DVE subs are emitted
                # late in the mm1 loop so they don't delay the relus there.
                for bs in range(4):
                    ln_part(bs)

            # ---- mm1 (+ mm2 interleaved one j-group behind for b0>0) ----
            hT_ts = []
            a_pss = [a_psp.tile([128, BB], f32, name="a_ps") for _ in range(2)]

            def mm2_partial(j):
                for dt_ in range(2):
                    nc.tensor.matmul(a_pss[dt_][:], W2_v[:, j, ts(dt_, 128)],
                                     hT_ts[j][:], start=(j == 0),
                                     stop=(j == 7), skip_group_check=True)

            if b0 == 0:
                # k-outer over j-groups of 6 then 2: the widest group keeps PE
                # ~90% busy while the W1/sT0 chunks stream in; first matmuls
                # need only W1 chunk 0 + the first sT quarter.
                for grp in (range(0, 6), range(6, 8)):
                    h_pss = [hd_psp.tile([128, BB], f32, name="h_ps",
                                         tag="ps") for _ in grp]
                    for k in range(8):
                        for gi, j in enumerate(grp):
                            nc.tensor.matmul(
                                h_pss[gi][:], W1_sb[:, k, ts(j, 128)],
                                sT_at(k), start=(k == 0), stop=(k == 7))
                    for gi, j in enumerate(grp):
                        hT_t = hT_pool.tile([128, BB], f32r, name="hT_t")
                        nc.vector.tensor_scalar(hT_t[:], h_pss[gi][:],
                                                b1_sb[:, j:j + 1], 0.0,
                                                OP.add, OP.max)
                        hT_ts.append(hT_t)
                for j in range(8):
                    mm2_partial(j)
            else:
                for j in range(8):
                    h_ps = hd_psp.tile([128, BB], f32, name="h_ps", tag="ps")
                    for k in range(8):
                        nc.tensor.matmul(h_ps[:], W1_sb[:, k, ts(j, 128)],
                                         sT_at(k), start=(k == 0),
                                         stop=(k == 7))
                    hT_t = hT_pool.tile([128, BB], f32r, name="hT_t")
                    nc.vector.tensor_scalar(hT_t[:], h_ps[:],
                                            b1_sb[:, j:j + 1], 0.0,
                                            OP.add, OP.max)
                    hT_ts.append(hT_t)
                    if 3 <= j <= 6:
                        gdn_part(j - 3)
                    if j >= 2:
                        mm2_partial(j - 2)
                mm2_partial(6)
                mm2_partial(7)

            awT_ts = []
            for dt_ in range(2):
                awT_t = awT_pool.tile([128, BB], f32r, name="awT_t")
                nc.vector.tensor_scalar_add(awT_t[:], a_pss[dt_][:],
                                            b2_sb[:, dt_:dt_ + 1])
                awT_ts.append(awT_t)

            if b0 == 0:
                for bs in range(4):
                    gumbel(bs)

            # ---- mm3 + compare/emit per 128-row group ----
            for bs in range(4):
                d_ps = hd_psp.tile([128, A_DIM], f32, name="d_ps", tag="ps")
                for dt_ in range(2):
                    nc.tensor.matmul(d_ps[:], awT_ts[dt_][:, ts(bs, 128)],
                                     Wd_v[:, dt_, :], start=(dt_ == 0),
                                     stop=(dt_ == 1 and not has_db))
                if has_db:
                    nc.tensor.matmul(d_ps[:], ones_sb[:], db_sb[:],
                                     start=False, stop=True)
                nc.vector.tensor_tensor(o_t[:, bw * 4 + bs, :], d_ps[:],
                                        gdn_ts[bs][:], OP.is_ge)
            last_of_group = (bw == 1) or (b0 == nb - 1)
            if last_of_group:
                nx = 4 * (bw + 1)
                if b0 == nb - 1:
                    # split the tail store so the last DMA starts earlier
                    nc.sync.dma_start(outv[:, g, 0:nx // 2, :],
                                      o_t[:, 0:nx // 2, :])
                    h2 = (nx // 2 + nx) // 2
                    nc.sync.dma_start(outv[:, g, nx // 2:h2, :],
                                      o_t[:, nx // 2:h2, :])
                    nc.sync.dma_start(outv[:, g, h2:nx, :],
                                      o_t[:, h2:nx, :])
                else:
                    nc.sync.dma_start(outv[:, g, 0:nx, :], o_t[:, 0:nx, :])
                cur_sT = next_sT

        if loop_iters is not None:
            loop_cm.__exit__(None, None, None)

    nc.compile()
    return nc


def kernel(s, u, W1, b1, W2, b2, head_W, head_b, _rpc=None):
    global LAST_EXEC_NS
    s = np.asarray(s, dtype=np.float32)
    u = np.asarray(u, dtype=np.float32)
    W1 = np.ascontiguousarray(np.asarray(W1, dtype=np.float32))
    W2 = np.ascontiguousarray(np.asarray(W2, dtype=np.float32))
    b1 = np.asarray(b1, dtype=np.float32)
    b2 = np.asarray(b2, dtype=np.float32)
    head_W = np.asarray(head_W, dtype=np.float32)
    head_b = np.asarray(head_b, dtype=np.float32)

    nrows = s.shape[0]
    rpc = _rpc if _rpc is not None else nrows // N_CORES
    assert nrows == rpc * N_CORES and rpc % BB == 0

    sT = np.ascontiguousarray(s.T)                      # [S_DIM, nrows]
    u2 = u.reshape(nrows, 2 * A_DIM)
    # permute u rows to (q, p, h) pair-major per core shard (done per core
    # below), pack W2/Wd partition-major
    Wd = np.ascontiguousarray((head_W[:, :, 0] - head_W[:, :, 1]).T)
    W2h = np.ascontiguousarray(
        W2.reshape(8, 128, D_HEAD).transpose(1, 0, 2)).reshape(128, 8 * D_HEAD)
    Wdh = np.ascontiguousarray(
        Wd.reshape(2, 128, A_DIM).transpose(1, 0, 2)).reshape(128, 2 * A_DIM)
    db = np.ascontiguousarray(head_b[:, 0] - head_b[:, 1]).reshape(1, A_DIM)
    has_db = bool(np.any(db))
    b1c = np.ascontiguousarray(b1.reshape(8, 128).T)
    b2c = np.ascontiguousarray(b2.reshape(2, 128).T)

    key = (rpc, has_db)
    if key not in _cache:
        _cache[key] = _build(rpc, has_db)
    nc = _cache[key]

    nq = rpc // 256
    in_maps = []
    for c in range(N_CORES):
        uc = u2[c * rpc:(c + 1) * rpc]
        up = np.ascontiguousarray(
            uc.reshape(nq, 2, 128, 2 * A_DIM).transpose(0, 2, 1, 3)
        ).reshape(rpc, 2 * A_DIM)
        m = {
            "sT": np.ascontiguousarray(sT[:, c * rpc:(c + 1) * rpc]),
            "u2": up,
            "W1d": W1, "W2d": W2h, "Wdd": Wdh, "b1d": b1c, "b2d": b2c,
        }
        if has_db:
            m["dbd"] = db
        in_maps.append(m)

    res = bass_utils.run_bass_kernel_spmd(nc, in_maps,
                                          core_ids=list(range(N_CORES)))
    LAST_EXEC_NS = res.exec_time_ns
    nb = rpc // BB
    xgrp = 8 if nb % 2 == 0 else 4
    shards = []
    for c in range(N_CORES):
        e = res.results[c]["out"]                        # [rpc, A_DIM] uint8
        # undo the (g, p, x) store permutation back to batch order
        e = e.reshape(rpc // (128 * xgrp), 128, xgrp, A_DIM)
        shards.append(e.transpose(0, 2, 1, 3).reshape(rpc, A_DIM))
    evens = np.concatenate(shards, axis=0)               # [nrows, A_DIM]
    out = np.empty((nrows, 2 * A_DIM), dtype=np.float32)
    ef = evens.astype(np.float32)
    out[:, 0::2] = ef
    out[:, 1::2] = 1.0 - ef
    return out

